# revision 1
# baseline (speedup 1.0000x reference)
"""TRN2 Bass kernel for nn_Dynamic_System: batched MLP Hessian/grad + 3x3 solve.

Math (per sample):
  L = T([td,sd]) + V([th,z]) with 2-hidden-layer tanh MLPs (HID=512).
  H = d2T/dtd2 (3x3), g = dV/dth (3), b_out = Bn([th,s,sDd]) (3)
  out = H^-1 (tau + b_out + g)

Analytic derivatives (feature-major layout [feat_part, batch_free]):
  T: h1=tanh(x@W1+b1), d1=1-h1^2, h2=tanh(h1@W2+b2), d2=1-h2^2
     v = W2 (d2*w3)             -> GEMM vs M_v[j,i]=w3[j]W2[i,j]
     e_raw = h1*d1*v            (H1[p] = sum_i -2*W1a[kp,i]W1a[lp,i] e_raw_i)
     P_k = (d1*W1a[k]) @ W2     -> GEMM vs W12_k[i,j]=W1[k,i]W2[i,j]
     c2 = -2*h2*d2*w3
     H2[kl] = sum_j c2 P_k P_l  (reduce via onehot-coeff matmuls)
  V: g = W1v[0:3] (d1v * (W2v (d2v*w3v)))
  Bn: plain forward.

Precision: T branch + H reduces in native fp32 (H errors are amplified by
cond(H) up to ~1e5; rhs errors are not). V/Bn branches in bf16.

Sharding: pure data parallel, batch 32768 -> 8 cores x 4096.
"""
import sys
import numpy as np

sys.path.insert(0, "/opt/trn_rl_repo")

import concourse.bass as bass
import concourse.bacc as bacc
import concourse.mybir as mybir
import concourse.tile as tile
from concourse.bass_utils import run_bass_kernel_spmd

F32 = mybir.dt.float32
BF16 = mybir.dt.bfloat16
AF = mybir.ActivationFunctionType
OP = mybir.AluOpType

B = 32768
NCORES = 8
BC = B // NCORES          # 4096 samples per core
HID = 512
NK = HID // 128           # 4 feature chunks
NBT = BC // 512           # 8 batch tiles of 512
PAIRS = [(0, 0), (0, 1), (0, 2), (1, 1), (1, 2), (2, 2)]

_PROGRAM = None
DEBUG = False


def _declare(nc):
    d = {}
    P = lambda n, sh, dt: nc.declare_dram_parameter(n, list(sh), dt, isOutput=False)
    # per-core data
    d["XT"] = P("XT", [6, BC], F32)
    d["XV"] = P("XV", [4, BC], BF16)
    d["XB"] = P("XB", [9, BC], BF16)
    d["TAUT"] = P("TAUT", [3, BC], F32)
    # T branch weights (fp32)
    d["TW1"] = P("TW1", [6, HID], F32)
    d["TB1"] = P("TB1", [128, NK], F32)
    d["W2T"] = P("W2T", [128, NK * HID], F32)
    d["TB2"] = P("TB2", [128, NK], F32)
    d["W12"] = P("W12", [3, 128, NK * HID], F32)
    d["MVT"] = P("MVT", [128, NK * HID], F32)
    d["W3N2"] = P("W3N2", [128, NK], F32)
    d["SPW"] = P("SPW", [3, 128, NK * 6], F32)
    d["COEF"] = P("COEF", [128, 6 * 6], F32)
    d["SEL"] = P("SEL", [128, 6], F32)
    # V branch (bf16 weights, fp32 biases)
    d["VW1"] = P("VW1", [4, HID], BF16)
    d["VB1"] = P("VB1", [128, NK], F32)
    d["W2V"] = P("W2V", [128, NK * HID], BF16)
    d["VB2"] = P("VB2", [128, NK], F32)
    d["MVV"] = P("MVV", [128, NK * HID], BF16)
    d["W1VA"] = P("W1VA", [128, NK * 3], BF16)
    # Bn branch
    d["BW1"] = P("BW1", [9, HID], BF16)
    d["BB1"] = P("BB1", [128, NK], F32)
    d["W2B"] = P("W2B", [128, NK * HID], BF16)
    d["BB2"] = P("BB2", [128, NK], F32)
    d["W3B"] = P("W3B", [128, NK * 3], BF16)
    d["B3B"] = P("B3B", [3, 1], F32)
    d["EYE9"] = P("EYE9", [9, 9], F32)
    d["OUT"] = nc.declare_dram_parameter("OUT", [128, BC // 128, 3], F32,
                                         isOutput=True)
    if DEBUG:
        d["DBG_H"] = nc.declare_dram_parameter("DBG_H", [6, BC], F32,
                                               isOutput=True)
        d["DBG_R"] = nc.declare_dram_parameter("DBG_R", [3, BC], F32,
                                               isOutput=True)
        d["DBG_S"] = nc.declare_dram_parameter("DBG_S", [128, BC // 128, 9],
                                               F32, isOutput=True)
    return d


def build_program():
    nc = bacc.Bacc()
    dp = _declare(nc)
    MM = nc.tensor.matmul

    with tile.TileContext(nc) as tc:
        # ---- persistent pools: weights, inputs, cross-phase sbuf ----
        wpool = tc.alloc_tile_pool(name="weights", bufs=1)
        w = {}
        w["TW1"] = wpool.tile([6, HID], F32, name="w_TW1")
        w["TB1"] = wpool.tile([128, NK], F32, name="w_TB1")
        w["W2T"] = wpool.tile([128, NK, HID], F32, name="w_W2T")
        w["TB2"] = wpool.tile([128, NK], F32, name="w_TB2")
        for k in range(3):
            w[f"W12_{k}"] = wpool.tile([128, NK, HID], F32, name=f"w_W12_{k}")
        w["MVT"] = wpool.tile([128, NK, HID], F32, name="w_MVT")
        w["W3N2"] = wpool.tile([128, NK], F32, name="w_W3N2")
        w["SPW"] = wpool.tile([128, 3, NK, 6], F32, name="w_SPW")
        w["COEF"] = wpool.tile([128, 6, 6], F32, name="w_COEF")
        w["SEL"] = wpool.tile([128, 6], F32, name="w_SEL")
        w["VW1"] = wpool.tile([4, HID], BF16, name="w_VW1")
        w["VB1"] = wpool.tile([128, NK], F32, name="w_VB1")
        w["W2V"] = wpool.tile([128, NK, HID], BF16, name="w_W2V")
        w["VB2"] = wpool.tile([128, NK], F32, name="w_VB2")
        w["MVV"] = wpool.tile([128, NK, HID], BF16, name="w_MVV")
        w["W1VA"] = wpool.tile([128, NK, 3], BF16, name="w_W1VA")
        w["BW1"] = wpool.tile([9, HID], BF16, name="w_BW1")
        w["BB1"] = wpool.tile([128, NK], F32, name="w_BB1")
        w["W2B"] = wpool.tile([128, NK, HID], BF16, name="w_W2B")
        w["BB2"] = wpool.tile([128, NK], F32, name="w_BB2")
        w["W3B"] = wpool.tile([128, NK, 3], BF16, name="w_W3B")
        w["B3B"] = wpool.tile([3, 1], F32, name="w_B3B")
        w["EYE9"] = wpool.tile([9, 9], F32, name="w_EYE9")
        # HSB: H entries (pair-major); RHSB: rhs vector
        RHSB = wpool.tile([3, BC], F32, name="RHSB")
        SOLVET1 = wpool.tile([128, 16, 9], F32, name="SOLVET1")
        SOLVET2 = wpool.tile([128, 16, 9], F32, name="SOLVET2")
        SOLV1 = wpool.tile([128, 16, 16], F32, name="SOLV1")
        SOLV2 = wpool.tile([128, 16, 16], F32, name="SOLV2")
        XOUT = wpool.tile([128, BC // 128, 3], F32, name="XOUT")

        # DMAs ordered by phase-T need; V/B weights go via gpsimd so the
        # sync queue reaches the critical ones immediately.
        for name in ("TW1", "TB1"):
            nc.sync.dma_start(w[name][:], dp[name][:])
        HH = NK * HID // 2
        nc.sync.dma_start(w["W2T"][:, 0:2, :], dp["W2T"][:, 0:HH])
        for k in range(3):
            nc.sync.dma_start(w[f"W12_{k}"][:, 0:2, :], dp["W12"][k, :, 0:HH])
        nc.sync.dma_start(w["W2T"][:, 2:4, :], dp["W2T"][:, HH:])
        for k in range(3):
            nc.sync.dma_start(w[f"W12_{k}"][:, 2:4, :], dp["W12"][k, :, HH:])
        for name in ("TB2", "W3N2"):
            nc.sync.dma_start(w[name][:], dp[name][:])
        nc.sync.dma_start(w["COEF"][:], dp["COEF"][:])
        nc.sync.dma_start(w["SEL"][:], dp["SEL"][:])
        nc.sync.dma_start(w["MVT"][:], dp["MVT"][:])
        for g3_ in range(3):
            nc.sync.dma_start(w["SPW"][:, g3_, :, :], dp["SPW"][g3_])
        for name in ("VW1", "VB1", "VB2", "BW1", "BB1", "BB2", "B3B", "EYE9"):
            nc.gpsimd.dma_start(w[name][:], dp[name][:])
        for name in ("W2V", "MVV", "W1VA", "W2B", "W3B"):
            nc.gpsimd.dma_start(w[name][:], dp[name][:])

        # ================= 3x3 solve (Cramer + one refinement) =================
        def emit_solve_q(SOLVET, q, scr):
            sub = slice(8 * (q % 2), 8 * (q % 2) + 8)
            t = scr[:, sub, :]
            S = lambda j: SOLVET[:, sub, j]
            a, b_, c_, dd, ee, ff = (S(j) for j in range(6))
            r0, r1, r2 = S(6), S(7), S(8)
            T_ = lambda j: t[:, :, j]
            tt = nc.vector.tensor_tensor
            tt(T_(0), dd, ff, OP.mult); tt(T_(6), ee, ee, OP.mult)
            tt(T_(0), T_(0), T_(6), OP.subtract)               # A0
            tt(T_(1), c_, ee, OP.mult); tt(T_(6), b_, ff, OP.mult)
            tt(T_(1), T_(1), T_(6), OP.subtract)               # A1
            tt(T_(2), b_, ee, OP.mult); tt(T_(6), c_, dd, OP.mult)
            tt(T_(2), T_(2), T_(6), OP.subtract)               # A2
            tt(T_(3), a, ff, OP.mult); tt(T_(6), c_, c_, OP.mult)
            tt(T_(3), T_(3), T_(6), OP.subtract)               # B1
            tt(T_(4), b_, c_, OP.mult); tt(T_(6), a, ee, OP.mult)
            tt(T_(4), T_(4), T_(6), OP.subtract)               # B2
            tt(T_(5), a, dd, OP.mult); tt(T_(6), b_, b_, OP.mult)
            tt(T_(5), T_(5), T_(6), OP.subtract)               # C2
            tt(T_(6), a, T_(0), OP.mult)
            tt(T_(7), b_, T_(1), OP.mult)
            tt(T_(6), T_(6), T_(7), OP.add)
            tt(T_(7), c_, T_(2), OP.mult)
            tt(T_(6), T_(6), T_(7), OP.add)
            nc.vector.reciprocal(T_(7), T_(6))                 # 1/det
            ADJ = ((0, 1, 2), (1, 3, 4), (2, 4, 5))
            X_ = lambda j: XOUT[:, q * 8:(q + 1) * 8, j]
            for j, (ca, cb, cc_) in enumerate(ADJ):
                tt(T_(8), T_(ca), r0, OP.mult)
                tt(T_(9), T_(cb), r1, OP.mult)
                tt(T_(8), T_(8), T_(9), OP.add)
                tt(T_(9), T_(cc_), r2, OP.mult)
                tt(T_(8), T_(8), T_(9), OP.add)
                tt(X_(j), T_(8), T_(7), OP.mult)
            R_ = (r0, r1, r2)
            HS = (a, b_, c_, dd, ee, ff)
            for j, (ha, hb, hc) in enumerate(ADJ):
                tt(T_(11), HS[ha], X_(0), OP.mult)
                tt(T_(12), HS[hb], X_(1), OP.mult)
                tt(T_(11), T_(11), T_(12), OP.add)
                tt(T_(12), HS[hc], X_(2), OP.mult)
                tt(T_(11), T_(11), T_(12), OP.add)
                tt(T_(8 + j), R_[j], T_(11), OP.subtract)
            for j, (ca, cb, cc_) in enumerate(ADJ):
                tt(T_(11), T_(ca), T_(8), OP.mult)
                tt(T_(12), T_(cb), T_(9), OP.mult)
                tt(T_(11), T_(11), T_(12), OP.add)
                tt(T_(12), T_(cc_), T_(10), OP.mult)
                tt(T_(11), T_(11), T_(12), OP.add)
                tt(T_(11), T_(11), T_(7), OP.mult)
                tt(X_(j), X_(j), T_(11), OP.add)


        BTS = 512  # batch tile size

        # ================= Phase T (fp32): Hessian =================
        with tc.tile_pool(name="sbT", bufs=1) as sbT, \
             tc.tile_pool(name="psT", bufs=1, space="PSUM") as psT:
            xt = sbT.tile([6, BC], F32, name="xt_T")
            nc.scalar.dma_start(xt[:], dp["XT"][:])
            for bt in range(NBT):
                bs = slice(bt * BTS, (bt + 1) * BTS)
                h1 = sbT.tile([128, NK, BTS], F32, tag="h1", bufs=2)
                d1 = sbT.tile([128, NK, BTS], F32, tag="d1", bufs=2)
                d2 = sbT.tile([128, NK, BTS], F32, tag="d2", bufs=2)

                # ---- layer 1 ----
                for mo in range(NK):
                    a1 = psT.tile([128, BTS], F32, tag="psA", bufs=3)
                    MM(a1[:], w["TW1"][:, mo * 128:(mo + 1) * 128],
                       xt[:, bs], start=True, stop=True,
                       skip_group_check=True)
                    nc.scalar.activation(h1[:, mo, :], a1[:], AF.Tanh,
                                         bias=w["TB1"][:, mo:mo + 1], scale=1.0)
                    hsq = sbT.tile([128, BTS], F32, tag="hsq", bufs=2)
                    nc.scalar.activation(hsq[:], h1[:, mo, :], AF.Square)
                    nc.vector.tensor_scalar(d1[:, mo, :], hsq[:], -1.0, 1.0,
                                            OP.mult, OP.add)

                # ---- layer 2 + tangents + H2 ----
                Hps = psT.tile([128, BTS], F32, tag="H", bufs=1)
                # zero the whole bank: the gather matmul reads all 128 rows
                # and rows outside the reduce windows must be finite zeros
                # regardless of stale device PSUM state (0 * NaN = NaN).
                nc.vector.memset(Hps[:], 0.0)
                HGRP = {0: 0, 4: 0, 1: 1, 5: 1, 2: 2, 3: 2}
                HFIRST = (0, 1, 2)
                for mo in range(NK):
                    a2 = psT.tile([128, BTS], F32, tag="psA", bufs=3)
                    P0 = psT.tile([128, BTS], F32, tag="P0", bufs=1)
                    P1 = psT.tile([128, BTS], F32, tag="P1", bufs=1)
                    P2 = psT.tile([128, BTS], F32, tag="P2", bufs=1)
                    Pp = [P0, P1, P2]
                    for ki in range(NK):
                        st, sp = ki == 0, ki == NK - 1
                        lsl = slice(mo * 128, (mo + 1) * 128)
                        MM(a2[:], w["W2T"][:, ki, lsl], h1[:, ki, :],
                           start=st, stop=sp, skip_group_check=True)
                        for k in range(3):
                            MM(Pp[k][:], w[f"W12_{k}"][:, ki, lsl], d1[:, ki, :],
                               start=st, stop=sp, skip_group_check=True)
                    h2b = sbT.tile([128, BTS], F32, tag="h2b", bufs=2)
                    nc.scalar.activation(h2b[:], a2[:], AF.Tanh,
                                         bias=w["TB2"][:, mo:mo + 1], scale=1.0)
                    h2sq = sbT.tile([128, BTS], F32, tag="h2sq", bufs=2)
                    nc.scalar.activation(h2sq[:], h2b[:], AF.Square)
                    nc.vector.tensor_scalar(d2[:, mo, :], h2sq[:], -1.0, 1.0,
                                            OP.mult, OP.add)
                    c2 = sbT.tile([128, BTS], F32, tag="c2", bufs=2)
                    nc.vector.scalar_tensor_tensor(
                        c2[:], h2b[:], w["W3N2"][:, mo:mo + 1], d2[:, mo, :],
                        OP.mult, OP.mult)
                    Q = []
                    for k in range(3):
                        qk = sbT.tile([128, BTS], F32, tag=f"q{k}", bufs=2)
                        nc.vector.tensor_tensor(qk[:], Pp[k][:], c2[:], OP.mult)
                        Q.append(qk)
                    qkls = []
                    for p, (k, l) in enumerate(PAIRS):
                        qkl = sbT.tile([128, BTS], F32, tag="qkl", bufs=7)
                        nc.vector.tensor_tensor(qkl[:], Q[k][:], Pp[l][:], OP.mult)
                        qkls.append(qkl)
                    for p in range(6):
                        g = HGRP[p]
                        MM(Hps[32 * g:32 * g + 6, :], w["COEF"][:, p, :],
                           qkls[p][:],
                           start=(mo == 0 and p in HFIRST), stop=False,
                           tile_position=(0, 32 * g), skip_group_check=True)

                # ---- backward v + e + H1 ----
                for mi in range(NK):
                    vps = psT.tile([128, BTS], F32, tag="psA", bufs=3)
                    for ko in range(NK):
                        MM(vps[:], w["MVT"][:, ko, mi * 128:(mi + 1) * 128],
                           d2[:, ko, :], start=(ko == 0), stop=(ko == NK - 1),
                           skip_group_check=True)
                    e = sbT.tile([128, BTS], F32, tag="e", bufs=2)
                    nc.vector.tensor_tensor(e[:], h1[:, mi, :], vps[:], OP.mult)
                    nc.vector.tensor_tensor(e[:], e[:], d1[:, mi, :], OP.mult)
                    for g3_ in range(3):
                        MM(Hps[32 * g3_:32 * g3_ + 6, :],
                           w["SPW"][:, g3_, mi, :], e[:],
                           start=False, stop=(mi == NK - 1),
                           tile_position=(0, 32 * g3_), skip_group_check=True)

                hgat = sbT.tile([128, BTS], F32, tag="hgat", bufs=2)
                nc.vector.tensor_copy(hgat[:], Hps[:])
                sps6 = psT.tile([128, 4, 6], F32, tag="sps6", bufs=1)
                for cc in range(4):
                    MM(sps6[:, cc, :], hgat[:, cc * 128:(cc + 1) * 128],
                       w["SEL"][:], start=True, stop=True, skip_group_check=True)
                STt = SOLVET1 if bt < 4 else SOLVET2
                offt = (bt % 2) * 4 + 8 * ((bt // 2) % 2)
                nc.vector.tensor_copy(STt[:, offt:offt + 4, 0:6], sps6[:])

        # ================= Phase V (bf16): gradient g =================
        with tc.tile_pool(name="sbV", bufs=1) as sbV, \
             tc.tile_pool(name="psV", bufs=1, space="PSUM") as psV:
            xv = sbV.tile([4, BC], BF16, name="xv_V")
            nc.scalar.dma_start(xv[:], dp["XV"][:])
            taut = sbV.tile([3, BC], F32, name="taut_V")
            nc.scalar.dma_start(taut[:], dp["TAUT"][:])
            xb = sbV.tile([9, BC], BF16, name="xb_B")
            nc.scalar.dma_start(xb[:], dp["XB"][:])

            def emit_vb_tail(bt):
                sps = psV.tile([128, 4, 3], F32, tag="psS", bufs=2,
                               name=f"sps_{bt}")
                for cc in range(4):
                    c = bt * 4 + cc
                    nc.tensor.transpose(sps[:, cc, :],
                                        RHSB[:, c * 128:(c + 1) * 128],
                                        w["EYE9"][0:3, 0:3])
                ST = SOLVET1 if bt < 4 else SOLVET2
                off = (bt % 2) * 4 + 8 * ((bt // 2) % 2)
                nc.vector.tensor_copy(ST[:, off:off + 4, 6:9], sps[:])
                if bt % 2 == 1:
                    q = bt // 2
                    scr = (SOLV1, SOLV1, SOLV2, SOLV2)[q]
                    emit_solve_q((SOLVET1, SOLVET1, SOLVET2, SOLVET2)[q],
                                 q, scr)

            for bt in range(NBT):
                bs = slice(bt * BTS, (bt + 1) * BTS)
                h1v = sbV.tile([128, NK, BTS], BF16, tag="h1v", bufs=2)
                d1v = sbV.tile([128, NK, BTS], BF16, tag="d1v", bufs=2)
                d2v = sbV.tile([128, NK, BTS], BF16, tag="d2v", bufs=2)
                gv = sbV.tile([128, NK, BTS], BF16, tag="gv", bufs=2)
                for mo in range(NK):
                    a1 = psV.tile([128, BTS], F32, tag="psA", bufs=3)
                    MM(a1[:], w["VW1"][:, mo * 128:(mo + 1) * 128],
                       xv[:, bs], start=True, stop=True,
                       skip_group_check=True)
                    nc.scalar.activation(h1v[:, mo, :], a1[:], AF.Tanh,
                                         bias=w["VB1"][:, mo:mo + 1], scale=1.0)
                    hsq = sbV.tile([128, BTS], F32, tag="hsqv", bufs=2)
                    nc.scalar.activation(hsq[:], h1v[:, mo, :], AF.Square)
                    nc.vector.tensor_scalar(d1v[:, mo, :], hsq[:], -1.0, 1.0,
                                            OP.mult, OP.add)
                for mo in range(NK):
                    a2 = psV.tile([128, BTS], F32, tag="psA", bufs=3)
                    for ki in range(NK):
                        MM(a2[:], w["W2V"][:, ki, mo * 128:(mo + 1) * 128],
                           h1v[:, ki, :], start=(ki == 0), stop=(ki == NK - 1),
                           skip_group_check=True)
                    h2v = sbV.tile([128, BTS], BF16, tag="h2v", bufs=2)
                    nc.scalar.activation(h2v[:], a2[:], AF.Tanh,
                                         bias=w["VB2"][:, mo:mo + 1], scale=1.0)
                    hsq2 = sbV.tile([128, BTS], F32, tag="hsqv2", bufs=2)
                    nc.scalar.activation(hsq2[:], h2v[:], AF.Square)
                    nc.vector.tensor_scalar(d2v[:, mo, :], hsq2[:], -1.0, 1.0,
                                            OP.mult, OP.add)
                for mi in range(NK):
                    vps = psV.tile([128, BTS], F32, tag="psA", bufs=3)
                    for ko in range(NK):
                        MM(vps[:], w["MVV"][:, ko, mi * 128:(mi + 1) * 128],
                           d2v[:, ko, :], start=(ko == 0), stop=(ko == NK - 1),
                           skip_group_check=True)
                    nc.vector.tensor_tensor(gv[:, mi, :], d1v[:, mi, :], vps[:],
                                            OP.mult)
                rps = psV.tile([3, BTS], F32, tag="psR", bufs=2)
                for ki in range(NK):
                    MM(rps[:], w["W1VA"][:, ki, :], gv[:, ki, :],
                       start=(ki == 0), stop=(ki == NK - 1),
                       skip_group_check=True)
                # rhs = g + b3 + tau
                nc.vector.scalar_tensor_tensor(RHSB[:, bs], rps[:],
                                               w["B3B"][:], taut[:, bs],
                                               OP.add, OP.add)
                # --- Bn branch (merged) ---
                h1b = sbV.tile([128, NK, BTS], BF16, tag="h1b", bufs=2)
                h2bb = sbV.tile([128, NK, BTS], BF16, tag="h2bb", bufs=2)
                for mo in range(NK):
                    a1b = psV.tile([128, BTS], F32, tag="psA", bufs=3)
                    MM(a1b[:], w["BW1"][:, mo * 128:(mo + 1) * 128],
                       xb[:, bs], start=True, stop=True,
                       skip_group_check=True)
                    nc.scalar.activation(h1b[:, mo, :], a1b[:], AF.Tanh,
                                         bias=w["BB1"][:, mo:mo + 1], scale=1.0)
                for mo in range(NK):
                    a2b = psV.tile([128, BTS], F32, tag="psA", bufs=3)
                    for ki in range(NK):
                        MM(a2b[:], w["W2B"][:, ki, mo * 128:(mo + 1) * 128],
                           h1b[:, ki, :], start=(ki == 0), stop=(ki == NK - 1),
                           skip_group_check=True)
                    nc.scalar.activation(h2bb[:, mo, :], a2b[:], AF.Tanh,
                                         bias=w["BB2"][:, mo:mo + 1], scale=1.0)
                rpsb = psV.tile([3, BTS], F32, tag="psR", bufs=2)
                for ki in range(NK):
                    MM(rpsb[:], w["W3B"][:, ki, :], h2bb[:, ki, :],
                       start=(ki == 0), stop=(ki == NK - 1),
                       skip_group_check=True)
                nc.vector.tensor_tensor(RHSB[:, bs], rpsb[:], RHSB[:, bs],
                                        OP.add)
                if bt > 0:
                    emit_vb_tail(bt - 1)
            emit_vb_tail(NBT - 1)

        nc.sync.dma_start(dp["OUT"][:], XOUT[:])
        if DEBUG:
            nc.sync.dma_start(dp["DBG_R"][:], RHSB[:])
            nc.sync.dma_start(dp["DBG_S"][:, 0:16, :], SOLVET1[:])
            nc.sync.dma_start(dp["DBG_S"][:, 16:32, :], SOLVET2[:])

        wpool.release()
    nc.compile()
    return nc


def _spw_matrices(TW1):
    """Per-window H1 reduce matrices: window g keeps only its pairs' columns."""
    f32 = np.float32
    Sp = np.stack([-2.0 * TW1[k] * TW1[l] for k, l in PAIRS]).astype(f32)  # [6,512]
    full = Sp.reshape(6, NK, 128).transpose(2, 1, 0).reshape(128, NK * 6)
    WIN = ((0, 4), (1, 5), (2, 3))
    out = np.zeros((3, 128, NK * 6), f32)
    for g, cols in enumerate(WIN):
        for ki in range(NK):
            for c in cols:
                out[g, :, ki * 6 + c] = full[:, ki * 6 + c]
    return np.ascontiguousarray(out)


def _sel_matrix():
    """[128, 6] gather matrix: H2 rows {0,4,33,37,66,67} + H1 rows 96-101."""
    S = np.zeros((128, 6), np.float32)
    rows = {0: 0, 4: 4, 1: 33, 5: 37, 2: 66, 3: 67}
    for p, r in rows.items():
        S[r, p] = 1.0
    return S


def _host_prep(inputs):
    """Build the shared weight blobs + per-core input maps."""
    import ml_dtypes
    f32 = np.float32
    bf16 = ml_dtypes.bfloat16
    g = lambda n: np.asarray(inputs[n], dtype=f32)

    TW1, TB1, TW2, TB2, TW3 = g("T_W1"), g("T_b1"), g("T_W2"), g("T_b2"), g("T_W3")
    VW1, VB1, VW2, VB2, VW3 = g("V_W1"), g("V_b1"), g("V_W2"), g("V_b2"), g("V_W3")
    BW1, BB1, BW2, BB2, BW3, BB3 = (g("Bn_W1"), g("Bn_b1"), g("Bn_W2"),
                                    g("Bn_b2"), g("Bn_W3"), g("Bn_b3"))
    w3 = TW3[:, 0]
    w3v = VW3[:, 0]
    # [512, X] -> [128, NK*X] (partition-major chunk layout, single DMA)
    chunk_rows = lambda M: np.ascontiguousarray(
        M.reshape(NK, 128, -1).transpose(1, 0, 2).reshape(128, -1))
    colvec = lambda v: np.ascontiguousarray(v.reshape(NK, 128).T)  # [128,NK]

    shared = {
        "TW1": TW1, "TB1": colvec(TB1), "TB2": colvec(TB2),
        "W2T": chunk_rows(TW2),
        "W12": np.stack([chunk_rows(TW2 * TW1[k][:, None]) for k in range(3)][:]),
        "MVT": chunk_rows(np.ascontiguousarray(TW2.T) * w3[:, None][:]),
        "W3N2": colvec(-2.0 * w3),
        "SPW": _spw_matrices(TW1),
        "COEF": np.ascontiguousarray(
            np.stack([np.tile(np.eye(6, dtype=f32)[p], (128, 1))
                      for p in range(6)]).transpose(1, 0, 2).reshape(128, 36)),
        "VW1": VW1.astype(bf16), "VB1": colvec(VB1), "VB2": colvec(VB2),
        "W2V": chunk_rows(VW2).astype(bf16),
        "MVV": chunk_rows(np.ascontiguousarray(VW2.T) * w3v[:, None]).astype(bf16),
        "W1VA": np.ascontiguousarray(
            VW1[0:3].T.reshape(NK, 128, 3).transpose(1, 0, 2)
            .reshape(128, NK * 3)).astype(bf16),
        "BW1": BW1.astype(bf16), "BB1": colvec(BB1), "BB2": colvec(BB2),
        "W2B": chunk_rows(BW2).astype(bf16),
        "W3B": chunk_rows(BW3).astype(bf16),
        "B3B": BB3.reshape(3, 1),
        "EYE9": np.eye(9, dtype=f32),
        "SEL": _sel_matrix(),
    }

    td, sd, th = g("theta_dot"), g("s_dot"), g("theta")
    z, s, sdd, tau = g("z"), g("s"), g("s_Ddot"), g("tau")
    xt = np.concatenate([td, sd], axis=1)          # [B,6]
    xv = np.concatenate([th, z], axis=1)           # [B,4]
    xb = np.concatenate([th, s, sdd], axis=1)      # [B,9]

    in_maps = []
    for c in range(NCORES):
        rs = slice(c * BC, (c + 1) * BC)
        m = dict(shared)
        m["XT"] = np.ascontiguousarray(xt[rs].T)
        m["XV"] = np.ascontiguousarray(xv[rs].T).astype(bf16)
        m["XB"] = np.ascontiguousarray(xb[rs].T).astype(bf16)
        m["TAUT"] = np.ascontiguousarray(tau[rs].T)
        in_maps.append(m)
    return in_maps


def run(inputs, trace=False, trace_kwargs=None):
    global _PROGRAM
    if _PROGRAM is None:
        _PROGRAM = build_program()
    in_maps = _host_prep(inputs)
    res = run_bass_kernel_spmd(_PROGRAM, in_maps, list(range(NCORES)),
                               trace=trace, **(trace_kwargs or {}))
    outs = []
    for c in range(NCORES):
        o = res.results[c]["OUT"]                   # [128, BC//128, 3]
        outs.append(o.transpose(1, 0, 2).reshape(BC, 3))
    full = np.concatenate(outs, axis=0)[..., None].astype(np.float32)
    return full, res


def kernel(**inputs):
    out, _ = run(inputs, trace=False)
    return out

